# revision 1
# baseline (speedup 1.0000x reference)
import os
import sys
import numpy as np

sys.path.insert(0, "/opt/trn_rl_repo")

N = 100000
E = 800000
IN, HID, KG = 43, 64, 32
H1, H3 = 4, 2
NEG = 0.2
NC = 8
NLOC = N // NC            # 12500
NGRP = (NLOC + 127) // 128  # 98
NPAD = NGRP * 128         # 12544
PADN = NC * NPAD          # 100352
CH = 7                    # groups per chunk
NCHUNK = NGRP // CH       # 14
XW = 44                   # padded x width for L1 pregather


def _seg_sum(vals, seg, n):
    out = np.zeros((n,) + vals.shape[1:], vals.dtype)
    np.add.at(out, seg, vals)
    return out


def _np_forward(ins):
    x = ins["x"].astype(np.float64)
    src = np.asarray(ins["edge_index"][0]).astype(np.int64)
    dst = np.asarray(ins["edge_index"][1]).astype(np.int64)
    f64 = lambda k: np.asarray(ins[k]).astype(np.float64)

    def gat(xf, W, asrc, adst, b, heads, el=None):
        h = (xf @ W).reshape(N, heads, HID)
        a_s = np.einsum("nhc,hc->nh", h, asrc)
        a_d = np.einsum("nhc,hc->nh", h, adst)
        e = a_s[src] + a_d[dst]
        if el is not None:
            e = e + el
        e = np.where(e > 0, e, NEG * e)
        m = np.full((N, heads), -np.inf)
        np.maximum.at(m, dst, e)
        m = np.where(np.isfinite(m), m, 0.0)
        ex = np.exp(e - m[dst])
        s = _seg_sum(ex, dst, N)
        alpha = ex / (s[dst] + 1e-16)
        out = _seg_sum(alpha[:, :, None] * h[src], dst, N)
        return out.mean(1) + b

    def gcn(xf, W, b):
        deg = np.bincount(dst, minlength=N).astype(np.float64) + 1.0
        dinv = deg ** -0.5
        h = xf @ W
        nrm = dinv[src] * dinv[dst]
        out = _seg_sum(nrm[:, None] * h[src], dst, N)
        return out + h * (dinv ** 2)[:, None] + b

    def bn(xf, g, b):
        mu = xf.mean(0)
        var = xf.var(0)
        return (xf - mu) / np.sqrt(var + 1e-5) * g + b

    elu = lambda v: np.where(v > 0, v, np.exp(np.minimum(v, 0)) - 1)
    sig = lambda v: 1.0 / (1.0 + np.exp(-v))

    kg_onehot = x[:, -KG:]
    kg_cls = np.argmax(kg_onehot, -1)
    same = (kg_cls[src] == kg_cls[dst]).astype(np.float64)
    he = (same * float(ins["same_bias"]))[:, None, None] * f64("gat1_We").reshape(1, H1, HID)
    el = np.einsum("ehc,hc->eh", he, f64("gat1_aedge"))

    xg = gat(x, f64("gat1_W"), f64("gat1_asrc"), f64("gat1_adst"), f64("gat1_b"), H1, el)
    prior = kg_onehot @ f64("kg_prior_W") + f64("kg_prior_b")
    gs = sig(float(ins["gate"]))
    h = (1 - gs) * xg + gs * prior
    skip = x @ f64("skip_W") + f64("skip_b")
    h = elu(bn(h, f64("bn1_g"), f64("bn1_b"))) + skip
    s2 = h
    h = gcn(h, f64("gcn2_W"), f64("gcn2_b"))
    h = elu(bn(h, f64("bn2_g"), f64("bn2_b"))) + s2
    s3 = h
    h = gat(h, f64("gat3_W"), f64("gat3_asrc"), f64("gat3_adst"), f64("gat3_b"), H3)
    h = elu(bn(h, f64("bn3_g"), f64("bn3_b"))) + s3
    s4 = h
    h = gcn(h, f64("gcn4_W"), f64("gcn4_b"))
    h = elu(bn(h, f64("bn4_g"), f64("bn4_b"))) + s4
    raw = np.maximum(h @ f64("mlp_W1") + f64("mlp_b1"), 0) @ f64("mlp_W2") + f64("mlp_b2")
    nv = kg_onehot @ f64("vuln")
    return sig(raw + sig(float(ins["vuln_scale"])) * nv)[:, 0].astype(np.float32)


# ----------------------------------------------------------------------------
# host-side prep
# ----------------------------------------------------------------------------
def _host_prep(ins):
    import ml_dtypes
    x = np.asarray(ins["x"], np.float32)
    src = np.asarray(ins["edge_index"][0]).astype(np.int64)
    dst = np.asarray(ins["edge_index"][1]).astype(np.int64)
    f32 = lambda k: np.asarray(ins[k], np.float32)

    kg_cls = np.argmax(x[:, -KG:], -1)
    same = (kg_cls[src] == kg_cls[dst]).astype(np.float32)
    gs = 1.0 / (1.0 + np.exp(-float(ins["gate"])))
    sv = 1.0 / (1.0 + np.exp(-float(ins["vuln_scale"])))

    W1 = f32("gat1_W").reshape(IN, H1, HID)
    ws1 = np.einsum("chk,hk->ch", W1, f32("gat1_asrc"))
    wd1 = np.einsum("chk,hk->ch", W1, f32("gat1_adst"))
    ch = float(ins["same_bias"]) * np.einsum("hk,hk->h", f32("gat1_We").reshape(H1, HID),
                                             f32("gat1_aedge"))
    as1 = x @ ws1
    ad1 = x @ wd1
    e1 = as1[src] + ad1[dst] + same[:, None] * ch[None, :]   # [E,4]

    deg = np.bincount(dst, minlength=N).astype(np.float32) + 1.0
    dinv = deg ** -0.5
    nrm = dinv[src] * dinv[dst]
    selfn = dinv * dinv

    b2 = float(np.asarray(ins["mlp_b2"]).reshape(-1)[0])
    gsrc = (src // NLOC) * NPAD + src % NLOC

    cores = []
    core_e = []
    for c in range(NC):
        sel = np.nonzero((dst >= c * NLOC) & (dst < (c + 1) * NLOC))[0]
        sel = sel[np.argsort(dst[sel], kind="stable")]
        core_e.append(sel)
    grp_counts = np.zeros((NC, NGRP), np.int64)
    for c in range(NC):
        dl = dst[core_e[c]] - c * NLOC
        grp_counts[c] = np.bincount(dl // 128, minlength=NGRP)
    B = int(np.ceil(grp_counts.max() / 128))

    for c in range(NC):
        sel = core_e[c]
        dl = dst[sel] - c * NLOC
        idx_a = np.zeros((NGRP, 128, B), np.int32)
        dc_a = np.full((NGRP, 128, B), 255.0, np.float32)
        e1_a = np.full((NGRP, 128, B, H1), -60.0, np.float32)
        nm_a = np.zeros((NGRP, 128, B), np.float32)
        xg_a = np.zeros((NGRP, 128, B, XW), np.float32)
        off = np.concatenate([[0], np.cumsum(grp_counts[c])])
        for gi in range(NGRP):
            eg = sel[off[gi]:off[gi + 1]]
            ne = len(eg)
            j = np.arange(ne)
            b_, p_ = j // 128, j % 128
            idx_a[gi, p_, b_] = gsrc[eg]
            dc_a[gi, p_, b_] = (dl[off[gi]:off[gi + 1]] - gi * 128).astype(np.float32)
            e1_a[gi, p_, b_] = e1[eg]
            nm_a[gi, p_, b_] = nrm[eg]
            xg_a[gi, p_, b_, :IN] = x[src[eg]]
        xT = np.zeros((IN, NPAD), np.float32)
        xT[:, :NLOC] = x[c * NLOC:(c + 1) * NLOC].T
        sn = np.zeros((NGRP, 128), np.float32)
        sn.reshape(-1)[:NLOC] = selfn[c * NLOC:(c + 1) * NLOC]
        vn = np.zeros((NGRP, 128), np.float32)
        vn.reshape(-1)[:NLOC] = b2 + sv * f32("vuln")[kg_cls[c * NLOC:(c + 1) * NLOC], 0]
        cores.append(dict(
            idx=np.ascontiguousarray(idx_a.transpose(1, 0, 2)).reshape(128, NGRP * B),
            dc=dc_a.reshape(NGRP * 128, B),
            e1=e1_a.reshape(NGRP * 128, B * H1),
            nrm=nm_a.reshape(NGRP * 128, B),
            xg=xg_a.reshape(NGRP * 128, B * XW).astype(ml_dtypes.bfloat16),
            xT=xT.astype(ml_dtypes.bfloat16), sn=sn.T.copy(), vn=vn.T.copy(),
        ))

    W3 = f32("gat3_W").reshape(HID, H3, HID)
    ws3 = np.einsum("chk,hk->ch", W3, f32("gat3_asrc"))
    wd3 = np.einsum("chk,hk->ch", W3, f32("gat3_adst"))

    shared = dict(
        iota=np.tile(np.arange(128, dtype=np.float32)[None, :], (128, 1)),
        ones=np.ones((128, 1), np.float32),
        onerow=np.ones((1, 128), np.float32),
        w1s=(np.concatenate([W1[:, h, :] for h in range(H1)], 0) * (1 - gs) / H1
             ).astype(np.float32),
        kgw=np.concatenate([np.zeros((IN - KG, HID), np.float32),
                            (gs * f32("kg_prior_W")).astype(np.float32)], 0
                           ).astype(ml_dtypes.bfloat16),
        skw=f32("skip_W").astype(ml_dtypes.bfloat16),
        skb=np.tile(f32("skip_b")[None, :] - 1.0, (128, 1)),
        g2w=f32("gcn2_W"), g4w=f32("gcn4_W"),
        w3s=(np.concatenate([W3[:, h, :] for h in range(H3)], 0) / H3).astype(np.float32),
        wsd3=np.concatenate([ws3, wd3], 1),
        mw1=f32("mlp_W1"),
        mb1=np.tile(f32("mlp_b1")[None, :], (128, 1)),
        mw2=f32("mlp_W2"),
        bng=np.stack([f32(f"bn{i}_g") for i in (1, 2, 3, 4)], 1),
        bnb=np.stack([f32(f"bn{i}_b") for i in (1, 2, 3, 4)], 1),
        negone=np.full((128, 64), -1.0, np.float32),
    )
    return cores, shared, B, b2


# ----------------------------------------------------------------------------
# device kernel
# ----------------------------------------------------------------------------
def _build(B, b2):
    from concourse import bass, bacc, tile, mybir
    from concourse.masks import make_identity
    F32 = mybir.dt.float32
    BF16 = mybir.dt.bfloat16
    I32 = mybir.dt.int32
    AF = mybir.ActivationFunctionType
    OP = mybir.AluOpType

    nc = bacc.Bacc("TRN2", target_bir_lowering=False, debug=False,
                   enable_asserts=False, num_devices=NC)

    def din(name, shape, dt=F32):
        return nc.dram_tensor(name, shape, dt, kind="ExternalInput").ap()

    idx_i = din("idx", [128, NGRP * B], I32)
    dc_i = din("dc", [NGRP * 128, B])
    e1_i = din("e1", [NGRP * 128, B * H1])
    nrm_i = din("nrm", [NGRP * 128, B])
    xg_i = din("xg", [NGRP * 128, B * XW], BF16)
    xT_i = din("xT", [IN, NPAD], BF16)
    sn_i = din("sn", [128, NGRP])
    vn_i = din("vn", [128, NGRP])
    iota_i = din("iota", [128, 128])
    ones_i = din("ones", [128, 1])
    onerow_i = din("onerow", [1, 128])
    w1s_i = din("w1s", [H1 * IN, 64])
    kgw_i = din("kgw", [IN, 64], BF16)
    skw_i = din("skw", [IN, 64], BF16)
    skb_i = din("skb", [128, 64])
    g2w_i = din("g2w", [64, 64])
    g4w_i = din("g4w", [64, 64])
    w3s_i = din("w3s", [H3 * 64, 64])
    wsd3_i = din("wsd3", [64, 4])
    mw1_i = din("mw1", [64, 32])
    mb1_i = din("mb1", [128, 32])
    mw2_i = din("mw2", [32, 1])
    bng_i = din("bng", [64, 4])
    bnb_i = din("bnb", [64, 4])
    neg1_i = din("negone", [128, 64])
    y_o = nc.dram_tensor("y", [NPAD, 1], F32, kind="ExternalOutput").ap()
    DBG = os.environ.get("GNN_DEBUG_DUMPS", "")
    if DBG:
        dbg_o = [nc.dram_tensor(f"dbg{i}", [128, NGRP * 64], F32, kind="ExternalOutput").ap()
                 for i in range(4)]

    with tile.TileContext(nc) as tc:
        with tc.tile_pool(name="cst", bufs=1) as cst, \
             tc.tile_pool(name="big", bufs=1) as big, \
             tc.tile_pool(name="wrk", bufs=4) as wrk, \
             tc.tile_pool(name="gxp", bufs=3) as gxp, \
             tc.tile_pool(name="ps", bufs=2, space="PSUM") as ps, \
             tc.tile_pool(name="dram", bufs=1, space="DRAM") as dram:

            def load(ap, shape, dt=F32, pool=cst, tag=None):
                if tag is None:
                    tag = f"cst_{ap.tensor.name}"
                t = pool.tile(shape, dt, tag=tag)
                nc.sync.dma_start(t[:], ap[:])
                return t

            ident = cst.tile([128, 128], F32)
            make_identity(nc, ident[:])
            iota = load(iota_i, [128, 128])
            ones = load(ones_i, [128, 1])
            onerow = load(onerow_i, [1, 128])
            xT = load(xT_i, [IN, NPAD], BF16)
            sn = load(sn_i, [128, NGRP])
            vn = load(vn_i, [128, NGRP])
            w1sa = cst.tile([128, 64], F32, tag="w1sa")
            nc.sync.dma_start(w1sa[:], w1s_i[0:128, :])
            w1sb = cst.tile([44, 64], F32, tag="w1sb")
            nc.sync.dma_start(w1sb[:], w1s_i[128:H1 * IN, :])
            kgw = load(kgw_i, [IN, 64], BF16)
            skw = load(skw_i, [IN, 64], BF16)
            skb = load(skb_i, [128, 64])
            g2w = load(g2w_i, [64, 64])
            g4w = load(g4w_i, [64, 64])
            w3s = load(w3s_i, [H3 * 64, 64])
            wsd3 = load(wsd3_i, [64, 4])
            mw1 = load(mw1_i, [64, 32])
            mb1 = load(mb1_i, [128, 32])
            mw2 = load(mw2_i, [32, 1])
            bng = load(bng_i, [64, 4])
            bnb = load(bnb_i, [64, 4])
            neg1 = load(neg1_i, [128, 64])

            hpre = big.tile([128, NGRP * 64], F32)
            h1a = big.tile([128, NGRP * 64], F32, tag="h1a")
            h2a = big.tile([128, NGRP * 64], F32, tag="h2a")
            h3a = big.tile([128, NGRP * 64], F32, tag="h3a")
            ad3sb = big.tile([128, NGRP * 2], F32, tag="ad3sb")
            idxsb = big.tile([128, NGRP * B], I32, tag="idxsb")
            nc.sync.dma_start(idxsb[:], idx_i[:])

            h1sh = dram.tile([NPAD, 64], F32)
            h1full = dram.tile([PADN, 64], F32)
            x3sh = dram.tile([NPAD, 66], F32)
            x3full = dram.tile([PADN, 66], F32)
            h3sh = dram.tile([NPAD, 64], F32)
            h3full = dram.tile([PADN, 64], F32)
            stin = dram.tile([64, 2], F32, tag="stin")
            stout = dram.tile([64, 2], F32, tag="stout")

            RG = [list(range(NC))]

            def gather_group(tab_ap, g, width):
                gx = gxp.tile([128, B * width], F32, tag=f"gx{width}")
                gxr = gx[:].rearrange("p (b e) -> p b e", e=width)
                for b in range(B):
                    nc.gpsimd.indirect_dma_start(
                        out=gxr[:, b, :], out_offset=None, in_=tab_ap,
                        in_offset=bass.IndirectOffsetOnAxis(
                            ap=idxsb[:, g * B + b:g * B + b + 1], axis=0))
                return gx

            def build_S(dct, b):
                S = wrk.tile([128, 128], F32, tag="S")
                nc.vector.tensor_tensor(out=S[:], in0=dct[:, b:b + 1].to_broadcast([128, 128]),
                                        in1=iota[:], op=OP.is_equal)
                return S

            def bn_stats_mm(h_sb, hsq_sb, g, st_sb):
                pst = ps.tile([128, 4], F32, tag="pden")
                nc.tensor.matmul(skip_group_check=True, out=pst[:64, 0:1], lhsT=h_sb, rhs=ones[:],
                                 start=True, stop=True)
                nc.tensor.matmul(skip_group_check=True, out=pst[:64, 1:2], lhsT=hsq_sb, rhs=ones[:],
                                 start=True, stop=True)
                if g == 0:
                    nc.vector.tensor_scalar_mul(out=st_sb[:], in0=pst[:64, 0:2], scalar1=1.0)
                else:
                    nc.vector.tensor_tensor(out=st_sb[:], in0=st_sb[:], in1=pst[:64, 0:2], op=OP.add)

            def bn_finalize(st_sb, li):
                nc.sync.dma_start(stin[:], st_sb[:])
                nc.gpsimd.collective_compute(
                    "AllReduce", mybir.AluOpType.add, replica_groups=RG,
                    ins=[stin.opt()], outs=[stout.opt()])
                sg = wrk.tile([64, 2], F32, tag="sg")
                nc.sync.dma_start(sg[:], stout[:])
                mu = wrk.tile([64, 1], F32, tag="mu")
                nc.scalar.activation(out=mu[:], in_=sg[:, 0:1], func=AF.Copy, scale=1.0 / N)
                var = wrk.tile([64, 1], F32, tag="var")
                nc.scalar.activation(out=var[:], in_=sg[:, 1:2], func=AF.Copy, scale=1.0 / N)
                musq = wrk.tile([64, 1], F32, tag="musq")
                nc.scalar.activation(out=musq[:], in_=mu[:], func=AF.Square)
                nc.vector.tensor_tensor(out=var[:], in0=var[:], in1=musq[:], op=OP.subtract)
                nc.vector.tensor_scalar_add(out=var[:], in0=var[:], scalar1=1e-5)
                sd = wrk.tile([64, 1], F32, tag="sd")
                nc.scalar.activation(out=sd[:], in_=var[:], func=AF.Sqrt)
                rst = wrk.tile([64, 1], F32, tag="rst")
                nc.vector.reciprocal(out=rst[:], in_=sd[:])
                ab = wrk.tile([64, 2], F32, tag="ab")
                nc.vector.tensor_tensor(out=ab[:, 0:1], in0=bng[:, li:li + 1], in1=rst[:], op=OP.mult)
                t = wrk.tile([64, 1], F32, tag="bt")
                nc.vector.tensor_tensor(out=t[:], in0=mu[:], in1=ab[:, 0:1], op=OP.mult)
                nc.vector.tensor_tensor(out=ab[:, 1:2], in0=bnb[:, li:li + 1], in1=t[:], op=OP.subtract)
                pt = ps.tile([128, 128], F32, tag="ptr")
                nc.tensor.transpose(out=pt[:1, :64], in_=ab[:, 0:1], identity=ident[:64, :64])
                ar = wrk.tile([1, 64], F32, tag="ar")
                nc.scalar.activation(out=ar[:], in_=pt[:1, :64], func=AF.Copy)
                pt2 = ps.tile([128, 128], F32, tag="ptr")
                nc.tensor.transpose(out=pt2[:1, :64], in_=ab[:, 1:2], identity=ident[:64, :64])
                br = wrk.tile([1, 64], F32, tag="br")
                nc.scalar.activation(out=br[:], in_=pt2[:1, :64], func=AF.Copy)
                pb = ps.tile([128, 128], F32, tag="ptr")
                nc.tensor.matmul(skip_group_check=True, out=pb[:, 0:64], lhsT=onerow[:], rhs=ar[:], start=True, stop=True)
                nc.tensor.matmul(skip_group_check=True, out=pb[:, 64:128], lhsT=onerow[:], rhs=br[:], start=True, stop=True)
                abc = wrk.tile([128, 64], F32, tag="abc")
                nc.scalar.activation(out=abc[:], in_=pb[:, 0:64], func=AF.Copy)
                bbc = wrk.tile([128, 64], F32, tag="bbc")
                nc.scalar.activation(out=bbc[:], in_=pb[:, 64:128], func=AF.Copy)
                return abc, bbc

            def elu_skip(z_sb, skip_ps, extra_sb, out_sb):
                m = wrk.tile([128, 64], F32, tag="elm")
                nc.vector.tensor_scalar_min(out=m[:], in0=z_sb, scalar1=0.0)
                ee = wrk.tile([128, 64], F32, tag="ele")
                nc.scalar.activation(out=ee[:], in_=m[:], func=AF.Exp)
                p = wrk.tile([128, 64], F32, tag="elp")
                nc.vector.tensor_scalar_max(out=p[:], in0=z_sb, scalar1=0.0)
                nc.vector.tensor_tensor(out=p[:], in0=p[:], in1=ee[:], op=OP.add)
                if skip_ps is not None:
                    nc.vector.tensor_tensor(out=p[:], in0=p[:], in1=skip_ps, op=OP.add)
                nc.vector.tensor_tensor(out=out_sb, in0=p[:], in1=extra_sb, op=OP.add)

            # ================= LAYER 1 (GAT, host-pregathered x) =================
            st1sb = cst.tile([64, 2], F32, tag="stsb1")
            for g in range(NGRP):
                xgt = gxp.tile([128, B * XW], BF16, tag="xgt")
                nc.sync.dma_start(xgt[:], xg_i[g * 128:(g + 1) * 128, :])
                dct = wrk.tile([128, B], F32, tag="dc")
                nc.sync.dma_start(dct[:], dc_i[g * 128:(g + 1) * 128, :])
                e1t = wrk.tile([128, B * H1], F32, tag="e1")
                nc.sync.dma_start(e1t[:], e1_i[g * 128:(g + 1) * 128, :])
                lr = wrk.tile([128, B * H1], F32, tag="lr")
                nc.vector.tensor_scalar(out=lr[:], in0=e1t[:], scalar1=0.2, scalar2=None, op0=OP.mult)
                nc.vector.tensor_tensor(out=lr[:], in0=lr[:], in1=e1t[:], op=OP.max)
                ex = wrk.tile([128, B * H1], F32, tag="ex")
                nc.scalar.activation(out=ex[:], in_=lr[:], func=AF.Exp)

                pden = ps.tile([128, 4], F32, tag="pden")
                pagg = ps.tile([128, H1 * IN], F32, tag="pagg")
                for b in range(B):
                    S = build_S(dct, b)
                    nc.tensor.matmul(skip_group_check=True, out=pden[:], lhsT=S[:], rhs=ex[:, b * 4:(b + 1) * 4],
                                     start=(b == 0), stop=(b == B - 1))
                    rc = wrk.tile([128, H1 * IN], F32, tag="rc")
                    gsl = xgt[:, b * XW:b * XW + IN]
                    for h in range(H1):
                        dst_sl = rc[:, h * IN:(h + 1) * IN]
                        sc = ex[:, b * 4 + h:b * 4 + h + 1]
                        if h % 2 == 0:
                            nc.scalar.activation(out=dst_sl, in_=gsl, func=AF.Copy, scale=sc)
                        else:
                            nc.vector.tensor_scalar_mul(out=dst_sl, in0=gsl, scalar1=sc)
                    nc.tensor.matmul(skip_group_check=True, out=pagg[:], lhsT=S[:], rhs=rc[:],
                                     start=(b == 0), stop=(b == B - 1))
                den = wrk.tile([128, 4], F32, tag="den")
                nc.vector.tensor_scalar_add(out=den[:], in0=pden[:], scalar1=1e-16)
                r = wrk.tile([128, 4], F32, tag="r")
                nc.vector.reciprocal(out=r[:], in_=den[:])
                agg = wrk.tile([128, H1 * IN], F32, tag="agg")
                for h in range(H1):
                    sl = slice(h * IN, (h + 1) * IN)
                    if h % 2 == 0:
                        nc.scalar.activation(out=agg[:, sl], in_=pagg[:, sl], func=AF.Copy,
                                             scale=r[:, h:h + 1])
                    else:
                        nc.vector.tensor_scalar_mul(out=agg[:, sl], in0=pagg[:, sl],
                                                    scalar1=r[:, h:h + 1])
                ptr = ps.tile([128, 128], F32, tag="ptr")
                nc.tensor.transpose(out=ptr[:], in_=agg[:, :128], identity=ident[:])
                t1 = wrk.tile([128, 128], F32, tag="t1")
                nc.scalar.activation(out=t1[:], in_=ptr[:], func=AF.Copy)
                ptr2 = ps.tile([128, 128], F32, tag="ptr")
                nc.tensor.transpose(out=ptr2[:44, :], in_=agg[:, 128:H1 * IN], identity=ident[:])
                t2 = wrk.tile([44, 128], F32, tag="t2")
                nc.scalar.activation(out=t2[:], in_=ptr2[:44, :], func=AF.Copy)
                ph = ps.tile([128, 64], F32, tag="ptr")
                nc.tensor.matmul(skip_group_check=True, out=ph[:], lhsT=t1[:], rhs=w1sa[:], start=True, stop=False)
                nc.tensor.matmul(skip_group_check=True, out=ph[:], lhsT=t2[:], rhs=w1sb[:], start=False, stop=False)
                nc.tensor.matmul(skip_group_check=True, out=ph[:], lhsT=xT[:, g * 128:(g + 1) * 128], rhs=kgw[:],
                                 start=False, stop=True)
                hg = hpre[:, g * 64:(g + 1) * 64]
                nc.scalar.activation(out=hg, in_=ph[:], func=AF.Copy)
                hsq = wrk.tile([128, 64], F32, tag="hsq")
                nc.scalar.activation(out=hsq[:], in_=ph[:], func=AF.Square)
                bn_stats_mm(hg, hsq[:], g, st1sb[:])

            if DBG:
                nc.sync.dma_start(dbg_o[0][:], hpre[:])
            abc, bbc = bn_finalize(st1sb[:], 0)
            for g in range(NGRP):
                z = wrk.tile([128, 64], F32, tag="z")
                nc.vector.tensor_tensor(out=z[:], in0=hpre[:, g * 64:(g + 1) * 64], in1=abc[:], op=OP.mult)
                nc.vector.tensor_tensor(out=z[:], in0=z[:], in1=bbc[:], op=OP.add)
                psk = ps.tile([128, 64], F32, tag="ptr")
                nc.tensor.matmul(skip_group_check=True, out=psk[:], lhsT=xT[:, g * 128:(g + 1) * 128], rhs=skw[:],
                                 start=True, stop=True)
                h1t = wrk.tile([128, 64], F32, tag="hout")
                elu_skip(z[:], psk[:], skb[:], h1t[:])
                nc.scalar.activation(out=h1a[:, g * 64:(g + 1) * 64], in_=h1t[:], func=AF.Copy)
                nc.sync.dma_start(h1sh[g * 128:(g + 1) * 128, :], h1t[:])
            nc.gpsimd.collective_compute("AllGather", mybir.AluOpType.bypass, replica_groups=RG,
                                         ins=[h1sh.opt()], outs=[h1full.opt()])
            if DBG:
                nc.sync.dma_start(dbg_o[1][:], h1a[:])

            # ================= LAYER 2 (GCN) =================
            st2sb = cst.tile([64, 2], F32, tag="stsb2")
            for g in range(NGRP):
                gx = gather_group(h1full[:], g, 64)
                dct = wrk.tile([128, B], F32, tag="dc")
                nc.sync.dma_start(dct[:], dc_i[g * 128:(g + 1) * 128, :])
                nt = wrk.tile([128, B], F32, tag="nt")
                nc.sync.dma_start(nt[:], nrm_i[g * 128:(g + 1) * 128, :])
                pagg = ps.tile([128, 64], F32, tag="pagg")
                for b in range(B):
                    S = build_S(dct, b)
                    rn = wrk.tile([128, 64], F32, tag="rn")
                    nc.scalar.activation(out=rn[:], in_=gx[:, b * 64:(b + 1) * 64], func=AF.Copy,
                                         scale=nt[:, b:b + 1])
                    nc.tensor.matmul(skip_group_check=True, out=pagg[:], lhsT=S[:], rhs=rn[:],
                                     start=(b == 0), stop=(b == B - 1))
                ts = wrk.tile([128, 64], F32, tag="ts")
                nc.scalar.activation(out=ts[:], in_=h1a[:, g * 64:(g + 1) * 64], func=AF.Copy,
                                     scale=sn[:, g:g + 1])
                nc.vector.tensor_tensor(out=ts[:], in0=ts[:], in1=pagg[:], op=OP.add)
                ptr = ps.tile([128, 128], F32, tag="ptr")
                nc.tensor.transpose(out=ptr[:64, :], in_=ts[:], identity=ident[:])
                tT = wrk.tile([64, 128], F32, tag="tT")
                nc.scalar.activation(out=tT[:], in_=ptr[:64, :], func=AF.Copy)
                ph = ps.tile([128, 64], F32, tag="ptr")
                nc.tensor.matmul(skip_group_check=True, out=ph[:], lhsT=tT[:], rhs=g2w[:], start=True, stop=True)
                hg = hpre[:, g * 64:(g + 1) * 64]
                nc.scalar.activation(out=hg, in_=ph[:], func=AF.Copy)
                hsq = wrk.tile([128, 64], F32, tag="hsq")
                nc.scalar.activation(out=hsq[:], in_=ph[:], func=AF.Square)
                bn_stats_mm(hg, hsq[:], g, st2sb[:])
            abc, bbc = bn_finalize(st2sb[:], 1)
            for g in range(NGRP):
                z = wrk.tile([128, 64], F32, tag="z")
                nc.vector.tensor_tensor(out=z[:], in0=hpre[:, g * 64:(g + 1) * 64], in1=abc[:], op=OP.mult)
                nc.vector.tensor_tensor(out=z[:], in0=z[:], in1=bbc[:], op=OP.add)
                h2t = wrk.tile([128, 64], F32, tag="hout")
                elu_skip(z[:], None, neg1[:], h2t[:])
                nc.vector.tensor_tensor(out=h2t[:], in0=h2t[:], in1=h1a[:, g * 64:(g + 1) * 64], op=OP.add)
                nc.scalar.activation(out=h2a[:, g * 64:(g + 1) * 64], in_=h2t[:], func=AF.Copy)
                stg = wrk.tile([128, 66], F32, tag="stg")
                nc.vector.tensor_copy(out=stg[:, 0:64], in_=h2t[:])
                ptr = ps.tile([128, 128], F32, tag="ptr")
                nc.tensor.transpose(out=ptr[:64, :], in_=h2t[:], identity=ident[:])
                h2T = wrk.tile([64, 128], F32, tag="tT")
                nc.scalar.activation(out=h2T[:], in_=ptr[:64, :], func=AF.Copy)
                psd = ps.tile([128, 4], F32, tag="ptr")
                nc.tensor.matmul(skip_group_check=True, out=psd[:], lhsT=h2T[:], rhs=wsd3[:], start=True, stop=True)
                nc.vector.tensor_copy(out=stg[:, 64:66], in_=psd[:, 0:2])
                nc.vector.tensor_copy(out=ad3sb[:, g * 2:(g + 1) * 2], in_=psd[:, 2:4])
                nc.sync.dma_start(x3sh[g * 128:(g + 1) * 128, :], stg[:])
            nc.gpsimd.collective_compute("AllGather", mybir.AluOpType.bypass, replica_groups=RG,
                                         ins=[x3sh.opt()], outs=[x3full.opt()])
            if DBG:
                nc.sync.dma_start(dbg_o[2][:], h2a[:])

            # ================= LAYER 3 (GAT, 2 heads) =================
            st3sb = cst.tile([64, 2], F32, tag="stsb3")
            for g in range(NGRP):
                gx = gather_group(x3full[:], g, 66)
                dct = wrk.tile([128, B], F32, tag="dc")
                nc.sync.dma_start(dct[:], dc_i[g * 128:(g + 1) * 128, :])
                gxr = gx[:].rearrange("p (b e) -> p b e", e=66)
                e3 = wrk.tile([128, B * H3], F32, tag="e3")
                e3r = e3[:].rearrange("p (b e) -> p b e", e=H3)
                pade = ps.tile([128, 2 * B], F32, tag="pade")
                for b in range(B):
                    S = build_S(dct, b)
                    pst_t = ps.tile([128, 128], F32, tag="ptr")
                    nc.tensor.transpose(out=pst_t[:], in_=S[:], identity=ident[:])
                    ST = wrk.tile([128, 128], F32, tag="ST")
                    nc.scalar.activation(out=ST[:], in_=pst_t[:], func=AF.Copy)
                    nc.tensor.matmul(skip_group_check=True, out=pade[:, b * 2:(b + 1) * 2], lhsT=ST[:],
                                     rhs=ad3sb[:, g * 2:(g + 1) * 2], start=True, stop=True)
                nc.vector.tensor_tensor(out=e3r[:, :, :], in0=gxr[:, :, 64:66],
                                        in1=pade[:].rearrange("p (b e) -> p b e", e=H3),
                                        op=OP.add)
                lr = wrk.tile([128, B * H3], F32, tag="lr3")
                nc.vector.tensor_scalar(out=lr[:], in0=e3[:], scalar1=0.2, scalar2=None, op0=OP.mult)
                nc.vector.tensor_tensor(out=lr[:], in0=lr[:], in1=e3[:], op=OP.max)
                nc.vector.tensor_scalar_min(out=lr[:], in0=lr[:], scalar1=30.0)
                ex = wrk.tile([128, B * H3], F32, tag="ex3")
                nc.scalar.activation(out=ex[:], in_=lr[:], func=AF.Exp)
                pden = ps.tile([128, 2], F32, tag="pden")
                pagg = ps.tile([128, H3 * 64], F32, tag="pagg")
                for b in range(B):
                    S = build_S(dct, b)
                    nc.tensor.matmul(skip_group_check=True, out=pden[:], lhsT=S[:], rhs=ex[:, b * 2:(b + 1) * 2],
                                     start=(b == 0), stop=(b == B - 1))
                    rc = wrk.tile([128, H3 * 64], F32, tag="rc")
                    gsl = gxr[:, b, 0:64]
                    for h in range(H3):
                        dst_sl = rc[:, h * 64:(h + 1) * 64]
                        sc = ex[:, b * 2 + h:b * 2 + h + 1]
                        if h % 2 == 0:
                            nc.scalar.activation(out=dst_sl, in_=gsl, func=AF.Copy, scale=sc)
                        else:
                            nc.vector.tensor_scalar_mul(out=dst_sl, in0=gsl, scalar1=sc)
                    nc.tensor.matmul(skip_group_check=True, out=pagg[:], lhsT=S[:], rhs=rc[:],
                                     start=(b == 0), stop=(b == B - 1))
                den = wrk.tile([128, 2], F32, tag="den")
                nc.vector.tensor_scalar_add(out=den[:], in0=pden[:], scalar1=1e-16)
                r = wrk.tile([128, 2], F32, tag="r")
                nc.vector.reciprocal(out=r[:], in_=den[:])
                agg = wrk.tile([128, H3 * 64], F32, tag="agg")
                for h in range(H3):
                    sl = slice(h * 64, (h + 1) * 64)
                    if h % 2 == 0:
                        nc.scalar.activation(out=agg[:, sl], in_=pagg[:, sl], func=AF.Copy,
                                             scale=r[:, h:h + 1])
                    else:
                        nc.vector.tensor_scalar_mul(out=agg[:, sl], in0=pagg[:, sl],
                                                    scalar1=r[:, h:h + 1])
                ptr = ps.tile([128, 128], F32, tag="ptr")
                nc.tensor.transpose(out=ptr[:], in_=agg[:], identity=ident[:])
                t1 = wrk.tile([128, 128], F32, tag="t1")
                nc.scalar.activation(out=t1[:], in_=ptr[:], func=AF.Copy)
                ph = ps.tile([128, 64], F32, tag="ptr")
                nc.tensor.matmul(skip_group_check=True, out=ph[:], lhsT=t1[:], rhs=w3s[:], start=True, stop=True)
                hg = hpre[:, g * 64:(g + 1) * 64]
                nc.scalar.activation(out=hg, in_=ph[:], func=AF.Copy)
                hsq = wrk.tile([128, 64], F32, tag="hsq")
                nc.scalar.activation(out=hsq[:], in_=ph[:], func=AF.Square)
                bn_stats_mm(hg, hsq[:], g, st3sb[:])
            abc, bbc = bn_finalize(st3sb[:], 2)
            for g in range(NGRP):
                z = wrk.tile([128, 64], F32, tag="z")
                nc.vector.tensor_tensor(out=z[:], in0=hpre[:, g * 64:(g + 1) * 64], in1=abc[:], op=OP.mult)
                nc.vector.tensor_tensor(out=z[:], in0=z[:], in1=bbc[:], op=OP.add)
                h3t = wrk.tile([128, 64], F32, tag="hout")
                elu_skip(z[:], None, neg1[:], h3t[:])
                nc.vector.tensor_tensor(out=h3t[:], in0=h3t[:], in1=h2a[:, g * 64:(g + 1) * 64], op=OP.add)
                nc.scalar.activation(out=h3a[:, g * 64:(g + 1) * 64], in_=h3t[:], func=AF.Copy)
                nc.sync.dma_start(h3sh[g * 128:(g + 1) * 128, :], h3t[:])
            nc.gpsimd.collective_compute("AllGather", mybir.AluOpType.bypass, replica_groups=RG,
                                         ins=[h3sh.opt()], outs=[h3full.opt()])
            if DBG:
                nc.sync.dma_start(dbg_o[3][:], h3a[:])

            # ================= LAYER 4 (GCN) =================
            st4sb = cst.tile([64, 2], F32, tag="stsb4")
            for g in range(NGRP):
                gx = gather_group(h3full[:], g, 64)
                dct = wrk.tile([128, B], F32, tag="dc")
                nc.sync.dma_start(dct[:], dc_i[g * 128:(g + 1) * 128, :])
                nt = wrk.tile([128, B], F32, tag="nt")
                nc.sync.dma_start(nt[:], nrm_i[g * 128:(g + 1) * 128, :])
                pagg = ps.tile([128, 64], F32, tag="pagg")
                for b in range(B):
                    S = build_S(dct, b)
                    rn = wrk.tile([128, 64], F32, tag="rn")
                    nc.scalar.activation(out=rn[:], in_=gx[:, b * 64:(b + 1) * 64], func=AF.Copy,
                                         scale=nt[:, b:b + 1])
                    nc.tensor.matmul(skip_group_check=True, out=pagg[:], lhsT=S[:], rhs=rn[:],
                                     start=(b == 0), stop=(b == B - 1))
                ts = wrk.tile([128, 64], F32, tag="ts")
                nc.scalar.activation(out=ts[:], in_=h3a[:, g * 64:(g + 1) * 64], func=AF.Copy,
                                     scale=sn[:, g:g + 1])
                nc.vector.tensor_tensor(out=ts[:], in0=ts[:], in1=pagg[:], op=OP.add)
                ptr = ps.tile([128, 128], F32, tag="ptr")
                nc.tensor.transpose(out=ptr[:64, :], in_=ts[:], identity=ident[:])
                tT = wrk.tile([64, 128], F32, tag="tT")
                nc.scalar.activation(out=tT[:], in_=ptr[:64, :], func=AF.Copy)
                ph = ps.tile([128, 64], F32, tag="ptr")
                nc.tensor.matmul(skip_group_check=True, out=ph[:], lhsT=tT[:], rhs=g4w[:], start=True, stop=True)
                hg = hpre[:, g * 64:(g + 1) * 64]
                nc.scalar.activation(out=hg, in_=ph[:], func=AF.Copy)
                hsq = wrk.tile([128, 64], F32, tag="hsq")
                nc.scalar.activation(out=hsq[:], in_=ph[:], func=AF.Square)
                bn_stats_mm(hg, hsq[:], g, st4sb[:])
            abc, bbc = bn_finalize(st4sb[:], 3)
            for g in range(NGRP):
                z = wrk.tile([128, 64], F32, tag="z")
                nc.vector.tensor_tensor(out=z[:], in0=hpre[:, g * 64:(g + 1) * 64], in1=abc[:], op=OP.mult)
                nc.vector.tensor_tensor(out=z[:], in0=z[:], in1=bbc[:], op=OP.add)
                h4t = wrk.tile([128, 64], F32, tag="h4t")
                h4g = h4t[:]
                elu_skip(z[:], None, neg1[:], h4g)
                nc.vector.tensor_tensor(out=h4g, in0=h4g, in1=h3a[:, g * 64:(g + 1) * 64], op=OP.add)
                ptr = ps.tile([128, 128], F32, tag="ptr")
                nc.tensor.transpose(out=ptr[:64, :], in_=h4g, identity=ident[:])
                h4T = wrk.tile([64, 128], F32, tag="tT")
                nc.scalar.activation(out=h4T[:], in_=ptr[:64, :], func=AF.Copy)
                pm = ps.tile([128, 32], F32, tag="ptr")
                nc.tensor.matmul(skip_group_check=True, out=pm[:], lhsT=h4T[:], rhs=mw1[:], start=True, stop=True)
                rl = wrk.tile([128, 32], F32, tag="rl")
                nc.vector.tensor_tensor(out=rl[:], in0=pm[:], in1=mb1[:], op=OP.add)
                nc.scalar.activation(out=rl[:], in_=rl[:], func=AF.Relu)
                ptr2 = ps.tile([128, 128], F32, tag="ptr")
                nc.tensor.transpose(out=ptr2[:32, :], in_=rl[:], identity=ident[:])
                rlT = wrk.tile([32, 128], F32, tag="rlT")
                nc.scalar.activation(out=rlT[:], in_=ptr2[:32, :], func=AF.Copy)
                pr = ps.tile([128, 1], F32, tag="ptr")
                nc.tensor.matmul(skip_group_check=True, out=pr[:], lhsT=rlT[:], rhs=mw2[:], start=True, stop=True)
                raw = wrk.tile([128, 1], F32, tag="raw")
                nc.vector.tensor_tensor(out=raw[:], in0=pr[:], in1=vn[:, g:g + 1], op=OP.add)
                yt = wrk.tile([128, 1], F32, tag="yt")
                nc.scalar.activation(out=yt[:], in_=raw[:], func=AF.Sigmoid)
                nc.sync.dma_start(y_o[g * 128:(g + 1) * 128, :], yt[:])

    nc.compile()
    return nc


_CACHE = {}


def _device_run(ins):
    from concourse import bass_utils
    cores, shared, B, b2 = _host_prep(ins)
    key = (B,)
    if key not in _CACHE:
        _CACHE[key] = _build(B, b2)
    nc = _CACHE[key]
    in_maps = []
    for c in range(NC):
        m = dict(shared)
        m.update(cores[c])
        in_maps.append(m)
    res = bass_utils.run_bass_kernel_spmd(nc, in_maps, core_ids=list(range(NC)))
    y = np.zeros(N, np.float32)
    for c in range(NC):
        y[c * NLOC:(c + 1) * NLOC] = res.results[c]["y"][:NLOC, 0]
    return y


def kernel(**inputs):
    if os.environ.get("GNN_FORCE_NUMPY"):
        return _np_forward(inputs)
    try:
        return _device_run(inputs)
    except Exception as exc:  # fall back to a correct host implementation
        sys.stderr.write(f"[kernel] device path failed ({exc!r}); numpy fallback\n")
        return _np_forward(inputs)

